# revision 20
# baseline (speedup 1.0000x reference)
"""Trainium2 Bass kernel for nn_Decoder_27419071218002 (pointer-network decoder
with sequential categorical sampling).

Strategy:
  - Data-parallel over batch B=128 across 8 NeuronCores (16 rows/core).
  - Loop-invariant pieces (ref = cc @ W_ref, h_bar, q0, gumbel noise for every
    step) are precomputed on host CPU jax — these are bitwise identical to what
    the reference computes there, and the sampling keys are data-independent so
    the per-step Gumbel noise can be fully precomputed.
  - The 1023-step sequential decode loop runs entirely on-chip in one Bass
    kernel per core: ref (8MB) stays resident in SBUF in [H=128 partitions,
    B*S free] layout; per step the scalar engine applies tanh(ref + q) with a
    per-partition bias, the PE reduces over H against v, and the vector engine
    does masking / gumbel argmax / logsumexp; the sampled row of cell_context
    is gathered back via indirect DMA to build the next query.
"""

import sys

if "/opt/trn_rl_repo" not in sys.path:
    sys.path.insert(0, "/opt/trn_rl_repo")

import numpy as np

B, S, E, H = 128, 1024, 128, 128
N_CORES = 8
BL = B // N_CORES  # 16 batch rows per core
N_STEPS = S - 1  # 1023
C_LOGIT = 10.0
NEG_INF = -1e9


# ---------------------------------------------------------------------------
# Host-side precompute (CPU jax — bitwise identical to the reference's
# loop-invariant computations and random draws)
# ---------------------------------------------------------------------------

def _host_precompute(cell_context, high_mask, init_w, Wc, bc, Wv, bv, W_ref, W_q,
                     n_steps=N_STEPS):
    import jax
    import jax.numpy as jnp

    cpu = jax.devices("cpu")[0]
    with jax.default_device(cpu):
        put = lambda x: jax.device_put(jnp.asarray(x), cpu)
        cc = put(cell_context)
        ref = jnp.einsum("bse,eh->bsh", cc, put(W_ref))  # [B, S, H]
        h_bar = jnp.mean(cc, axis=1) @ put(Wc) + put(bc)  # [B, E]
        q0 = h_bar + (put(init_w) @ put(Wv) + put(bv))  # [B, E]
        q0p = q0 @ put(W_q)  # projected first-step query [B, H]

        base_key = jax.random.key(42)

        @jax.jit
        def gum(i):
            return jax.random.gumbel(
                jax.random.fold_in(base_key, i), (B, S), jnp.float32
            )

        g = jax.lax.map(gum, jnp.arange(n_steps))  # [n_steps, B, S]

        ref = np.asarray(ref)
        h_bar = np.asarray(h_bar)
        q0p = np.asarray(q0p)
        g = np.asarray(g)

    mask0 = np.asarray(high_mask).copy()
    mask0[:, 0] = 1.0
    maskadd0 = np.where(mask0 > 0, np.float32(NEG_INF), np.float32(0.0)).astype(
        np.float32
    )
    return ref, h_bar, q0p, g, maskadd0


# ---------------------------------------------------------------------------
# Bass kernel builder
# ---------------------------------------------------------------------------

def build_bass_kernel(n_steps=N_STEPS, strip=()):
    strip = set(strip)
    import concourse.bass as bass
    import concourse.bacc as bacc
    import concourse.mybir as mybir
    from concourse.bass import ds
    from concourse.masks import make_identity
    from concourse.tile import TileContext

    dt = mybir.dt
    AF = mybir.ActivationFunctionType
    OP = mybir.AluOpType
    f32 = dt.float32

    nc = bacc.Bacc(
        "TRN2", target_bir_lowering=False, debug=False, num_devices=N_CORES
    )

    # --- DRAM I/O ---
    refT_d = nc.declare_dram_parameter("refT", [H, BL * S], f32, isOutput=False)
    g_d = nc.declare_dram_parameter("g_all", [n_steps * BL, S], f32, isOutput=False)
    cc_d = nc.declare_dram_parameter("cc_rows", [BL * S, E], f32, isOutput=False)
    qT0_d = nc.declare_dram_parameter("qT0", [H, BL], f32, isOutput=False)
    hbarT_d = nc.declare_dram_parameter("h_barT", [E, BL], f32, isOutput=False)
    bvT_d = nc.declare_dram_parameter("bvT", [E, 1], f32, isOutput=False)
    Wv1_d = nc.declare_dram_parameter("Wv1", [E, E], f32, isOutput=False)
    Wv2_d = nc.declare_dram_parameter("Wv2", [E, E], f32, isOutput=False)
    Wq_d = nc.declare_dram_parameter("Wq", [E, H], f32, isOutput=False)
    v_d = nc.declare_dram_parameter("v_col", [H, 1], f32, isOutput=False)
    maskadd0_d = nc.declare_dram_parameter("maskadd0", [BL, S], f32, isOutput=False)
    iota_d = nc.declare_dram_parameter("iota_s", [BL, S], f32, isOutput=False)
    bbase_d = nc.declare_dram_parameter("bbase", [BL, 1], dt.uint32, isOutput=False)
    pidx_d = nc.declare_dram_parameter("pidx", [BL, 1], f32, isOutput=False)
    idx_out_d = nc.declare_dram_parameter("idx_out", [BL, n_steps], dt.uint32,
                                          isOutput=True)
    logp_out_d = nc.declare_dram_parameter("logp_out", [BL, n_steps], f32,
                                           isOutput=True)

    with TileContext(nc) as tc:
        from contextlib import ExitStack

        with ExitStack() as ctx:
            const = ctx.enter_context(tc.tile_pool(name="const", bufs=1))
            work = ctx.enter_context(tc.tile_pool(name="work", bufs=1))
            tpool = ctx.enter_context(tc.tile_pool(name="tch", bufs=2))
            upsum = ctx.enter_context(tc.tile_pool(name="upsum", bufs=2, space="PSUM"))
            spsum = ctx.enter_context(tc.tile_pool(name="spsum", bufs=1, space="PSUM"))

            # --- persistent SBUF tensors ---
            ref_sb = const.tile([H, BL * S], f32, tag="ref")
            Wv1_sb = const.tile([E, E], f32, tag="wv1")
            Wv2_sb = const.tile([E, E], f32, tag="wv2")
            Wq_sb = const.tile([E, H], f32, tag="wq")
            v_sb = const.tile([H, 1], f32, tag="v")
            v_rep_sb = const.tile([H, 128], f32, tag="vrep")
            bvT_sb = const.tile([E, 1], f32, tag="bvt")
            hbarT_sb = const.tile([E, BL], f32, tag="hbart")
            iota_sb = const.tile([BL, S], f32, tag="iota")
            bbase_sb = const.tile([BL, 1], dt.uint32, tag="bbase")
            pidx_sb = const.tile([BL, 1], f32, tag="pidx")
            rmask_sb = const.tile([BL, BL], dt.uint8, tag="rmask")
            ident_sb = const.tile([128, 128], f32, tag="ident")

            maskadd_sb = const.tile([BL, S], f32, tag="maskadd")
            qT_sb = const.tile([H, BL], f32, tag="qt")
            queryT_sb = const.tile([E, BL], f32, tag="queryt")
            inithT_sb = const.tile([E, BL], f32, tag="initht")
            hT_sb = const.tile([E, BL], f32, tag="ht")

            # --- per-step scratch (allocated once, reused each iteration) ---
            g_sb = work.tile([BL, S], f32, tag="g")
            u_sb = work.tile([BL, S], f32, tag="u")
            th_sb = work.tile([BL, S], f32, tag="th")
            logits_sb = work.tile([BL, S], f32, tag="logits")
            y_sb = work.tile([BL, S], f32, tag="y")
            exp_sb = work.tile([BL, S], f32, tag="exp")
            eq_sb = work.tile([BL, S], f32, tag="eq")
            junk_sb = work.tile([BL, S], f32, tag="junk")
            max8_sb = work.tile([BL, 8], f32, tag="max8")
            idx8_sb = work.tile([BL, 8], dt.uint32, tag="idx8")
            m_sb = work.tile([BL, 1], f32, tag="m")
            negm_sb = work.tile([BL, 1], f32, tag="negm")
            sumexp_sb = work.tile([BL, 1], f32, tag="sumexp")
            lse_sb = work.tile([BL, 1], f32, tag="lse")
            usel_sb = work.tile([BL, 1], f32, tag="usel")
            logp_sb = work.tile([BL, 1], f32, tag="logp")
            idxf_sb = work.tile([BL, 1], f32, tag="idxf")
            idxg_sb = work.tile([BL, 1], dt.uint32, tag="idxg")
            h_rows_sb = work.tile([BL, E], f32, tag="hrows")

            # --- init loads ---
            nc.sync.dma_start(ref_sb[:], refT_d[:])
            nc.sync.dma_start(Wv1_sb[:], Wv1_d[:])
            nc.sync.dma_start(Wv2_sb[:], Wv2_d[:])
            nc.sync.dma_start(Wq_sb[:], Wq_d[:])
            nc.sync.dma_start(v_sb[:], v_d[:])
            nc.sync.dma_start(bvT_sb[:], bvT_d[:])
            nc.sync.dma_start(hbarT_sb[:], hbarT_d[:])
            nc.sync.dma_start(iota_sb[:], iota_d[:])
            nc.sync.dma_start(bbase_sb[:], bbase_d[:])
            nc.sync.dma_start(maskadd_sb[:], maskadd0_d[:])
            nc.sync.dma_start(qT_sb[:], qT0_d[:])
            make_identity(nc, ident_sb[:])
            nc.sync.dma_start(pidx_sb[:], pidx_d[:])
            nc.vector.tensor_copy(v_rep_sb[:], v_sb[:, 0:1].to_broadcast([H, 128]))
            for b in range(BL):
                nc.vector.tensor_scalar(
                    rmask_sb[:, b:b + 1], pidx_sb[:], float(b), None,
                    op0=OP.is_equal,
                )
            # Collapse the init-DMA fan-in to one semaphore: engine
            # instructions (ACT in particular) only have a single wait slot.
            tc.strict_bb_all_engine_barrier()

            def body(i, first):
                if "dyn" in strip and not isinstance(i, int):
                    i = 0
                # Gumbel slab for this step
                nc.sync.dma_start(g_sb[:], g_d[ds(i * BL, BL), :])

                # --- big stage: t = tanh(ref + q), u = sum_h v*t ---
                for b in range(BL):
                    tch = tpool.tile([H, S], f32, tag="t")
                    nc.scalar.activation(
                        tch[:],
                        ref_sb[:, b * S:(b + 1) * S],
                        AF.Tanh,
                        bias=qT_sb[:, b:b + 1],
                        scale=1.0,
                    )
                    ups = upsum.tile([128, S], f32, tag="ups")
                    nc.tensor.matmul(
                        ups[:, 0:512], lhsT=v_rep_sb[:], rhs=tch[:, 0:512],
                        start=True, stop=True,
                    )
                    nc.tensor.matmul(
                        ups[:, 512:1024], lhsT=v_rep_sb[:], rhs=tch[:, 512:1024],
                        start=True, stop=True,
                    )
                    # u is replicated across all PSUM partitions; a predicated
                    # copy lands it on row b only (DVE slices must start at a
                    # 32-aligned partition, so a direct [b:b+1] copy is out).
                    nc.vector.copy_predicated(
                        u_sb[:],
                        rmask_sb[:, b:b + 1].to_broadcast([BL, S]),
                        ups[0:BL, :],
                    )

                # --- logits = C*tanh(u) + maskadd ---
                nc.scalar.activation(th_sb[:], u_sb[:], AF.Tanh)
                nc.vector.scalar_tensor_tensor(
                    logits_sb[:], in0=th_sb[:], scalar=C_LOGIT, in1=maskadd_sb[:],
                    op0=OP.mult, op1=OP.add,
                )

                # --- gumbel argmax ---
                nc.vector.tensor_tensor(y_sb[:], logits_sb[:], g_sb[:], op=OP.add)
                if "max" in strip:
                    nc.vector.memset(max8_sb[:], 0.0)
                    nc.vector.memset(idx8_sb[:], 0)
                else:
                    nc.vector.max(max8_sb[:], y_sb[:])
                    nc.vector.max_index(idx8_sb[:], max8_sb[:], y_sb[:])

                # --- logp = logits[idx] - m - log(sum(exp(logits - m))) ---
                nc.vector.tensor_reduce(
                    m_sb[:], logits_sb[:], axis=mybir.AxisListType.X, op=OP.max
                )
                nc.vector.tensor_scalar_mul(negm_sb[:], m_sb[:], -1.0)
                nc.scalar.activation(
                    exp_sb[:], logits_sb[:], AF.Exp, bias=negm_sb[:, 0:1], scale=1.0,
                    accum_out=sumexp_sb[:],
                )
                nc.scalar.activation(lse_sb[:], sumexp_sb[:], AF.Ln)

                # one-hot of idx (f32 compare against iota)
                nc.vector.tensor_copy(idxf_sb[:], idx8_sb[:, 0:1])
                nc.vector.tensor_scalar(
                    eq_sb[:], iota_sb[:], idxf_sb[:, 0:1], None, op0=OP.is_equal
                )
                # select logits[idx] via masked sum (NOTE: tensor_tensor_reduce
                # is a custom-DVE op whose ucode table this runtime does not
                # load — it crashes the exec unit; use two standard ops)
                nc.vector.tensor_tensor(
                    junk_sb[:], logits_sb[:], eq_sb[:], op=OP.mult
                )
                nc.vector.tensor_reduce(
                    usel_sb[:], junk_sb[:], axis=mybir.AxisListType.X, op=OP.add
                )
                nc.vector.tensor_scalar(
                    logp_sb[:], usel_sb[:], m_sb[:, 0:1], lse_sb[:, 0:1],
                    op0=OP.subtract, op1=OP.subtract,
                )

                # --- outputs ---
                nc.sync.dma_start(idx_out_d[:, ds(i, 1)], idx8_sb[:, 0:1])
                nc.sync.dma_start(logp_out_d[:, ds(i, 1)], logp_sb[:, 0:1])

                # --- mask update (for next step) ---
                nc.vector.scalar_tensor_tensor(
                    maskadd_sb[:], in0=eq_sb[:], scalar=NEG_INF, in1=maskadd_sb[:],
                    op0=OP.mult, op1=OP.add,
                )

                # --- gather h = cc[b, idx_b, :] and build next query ---
                nc.vector.tensor_tensor(
                    idxg_sb[:], idx8_sb[:, 0:1], bbase_sb[:], op=OP.add
                )
                if "ind" in strip:
                    nc.sync.dma_start(h_rows_sb[:], cc_d[0:BL, :])
                else:
                    nc.gpsimd.indirect_dma_start(
                        out=h_rows_sb[:],
                        out_offset=None,
                        in_=cc_d[:],
                        in_offset=bass.IndirectOffsetOnAxis(
                            ap=idxg_sb[:, :1], axis=0
                        ),
                    )
                hT_ps = spsum.tile([E, BL], f32, tag="sp")
                nc.tensor.transpose(hT_ps[:], h_rows_sb[:], ident_sb[:BL, :BL])
                nc.vector.tensor_copy(hT_sb[:], hT_ps[:])
                if first:
                    nc.vector.tensor_copy(inithT_sb[:], hT_sb[:])

                q_ps = spsum.tile([E, BL], f32, tag="sp")
                nc.tensor.matmul(
                    q_ps[:], lhsT=Wv1_sb[:], rhs=inithT_sb[:], start=True, stop=False
                )
                nc.tensor.matmul(
                    q_ps[:], lhsT=Wv2_sb[:], rhs=hT_sb[:], start=False, stop=True
                )
                nc.vector.tensor_tensor(
                    queryT_sb[:], q_ps[:], hbarT_sb[:], op=OP.add
                )
                nc.vector.tensor_scalar(
                    queryT_sb[:], queryT_sb[:], bvT_sb[:, 0:1], None, op0=OP.add
                )
                qT_ps = spsum.tile([H, BL], f32, tag="sp")
                nc.tensor.matmul(
                    qT_ps[:], lhsT=Wq_sb[:], rhs=queryT_sb[:], start=True, stop=True
                )
                nc.vector.tensor_copy(qT_sb[:], qT_ps[:])

            body(0, True)
            if n_steps > 1:
                with tc.For_i(1, n_steps) as iv:
                    body(iv, False)

    nc.compile()
    return nc


# ---------------------------------------------------------------------------
# Per-core input maps
# ---------------------------------------------------------------------------

def make_in_maps(cell_context, ref, h_bar, q0p, g, maskadd0, Wv, bv, W_q, v,
                 n_steps=N_STEPS):
    cc = np.ascontiguousarray(np.asarray(cell_context), dtype=np.float32)
    Wv = np.asarray(Wv, dtype=np.float32)
    bv = np.asarray(bv, dtype=np.float32)
    W_q = np.asarray(W_q, dtype=np.float32)
    v = np.asarray(v, dtype=np.float32)

    iota_s = np.broadcast_to(
        np.arange(S, dtype=np.float32)[None, :], (BL, S)
    ).copy()
    pidx = np.arange(BL, dtype=np.float32)[:, None].copy()
    bbase = (np.arange(BL, dtype=np.uint32) * S)[:, None].copy()
    Wv1 = np.ascontiguousarray(Wv[:E])
    Wv2 = np.ascontiguousarray(Wv[E:])
    bvT = np.ascontiguousarray(bv[:, None])
    v_col = np.ascontiguousarray(v[:, None])

    in_maps = []
    for c in range(N_CORES):
        sl = slice(c * BL, (c + 1) * BL)
        refT = np.ascontiguousarray(
            np.transpose(ref[sl], (2, 0, 1)).reshape(H, BL * S)
        )
        in_maps.append(
            {
                "refT": refT,
                "g_all": np.ascontiguousarray(
                    g[:n_steps, sl, :].reshape(n_steps * BL, S)
                ),
                "cc_rows": np.ascontiguousarray(cc[sl].reshape(BL * S, E)),
                "qT0": np.ascontiguousarray(q0p[sl].T),
                "h_barT": np.ascontiguousarray(h_bar[sl].T),
                "bvT": bvT,
                "Wv1": Wv1,
                "Wv2": Wv2,
                "Wq": W_q,
                "v_col": v_col,
                "maskadd0": np.ascontiguousarray(maskadd0[sl]),
                "iota_s": iota_s,
                "bbase": bbase,
                "pidx": pidx,
            }
        )
    return in_maps


_BUILD_CACHE = {}


def _get_kernel(n_steps=N_STEPS):
    if n_steps not in _BUILD_CACHE:
        _BUILD_CACHE[n_steps] = build_bass_kernel(n_steps)
    return _BUILD_CACHE[n_steps]


def run_on_hw(in_maps, n_steps=N_STEPS, trace=False):
    from concourse.bass_utils import run_bass_kernel_spmd

    nc = _get_kernel(n_steps)
    res = run_bass_kernel_spmd(
        nc, in_maps, core_ids=list(range(len(in_maps))), trace=trace
    )
    return res


def kernel(node_context, original_data, cell_context, high_mask, low_mask,
           init_w, Wc, bc, Wv, bv, W_ref, W_q, v):
    cell_context = np.asarray(cell_context)
    ref, h_bar, q0p, g, maskadd0 = _host_precompute(
        cell_context, high_mask, init_w, Wc, bc, Wv, bv, W_ref, W_q
    )
    in_maps = make_in_maps(
        cell_context, ref, h_bar, q0p, g, maskadd0, Wv, bv, W_q, v
    )
    res = run_on_hw(in_maps)
    logps = np.concatenate([r["logp_out"] for r in res.results], axis=0)
    idxs = np.concatenate(
        [r["idx_out"].astype(np.int32) for r in res.results], axis=0
    )
    return logps, idxs


# revision 22
# speedup vs baseline: 1.2420x; 1.2420x over previous
"""Trainium2 Bass kernel for nn_Decoder_27419071218002 (pointer-network decoder
with sequential categorical sampling).

Strategy:
  - Data-parallel over batch B=128 across 8 NeuronCores (16 rows/core).
  - Loop-invariant pieces (ref = cc @ W_ref, h_bar, q0, gumbel noise for every
    step) are precomputed on host CPU jax — these are bitwise identical to what
    the reference computes there, and the sampling keys are data-independent so
    the per-step Gumbel noise can be fully precomputed.
  - The 1023-step sequential decode loop runs entirely on-chip in one Bass
    kernel per core: ref (8MB) stays resident in SBUF in [H=128 partitions,
    B*S free] layout; per step the scalar engine applies tanh(ref + q) with a
    per-partition bias, the PE reduces over H against v, and the vector engine
    does masking / gumbel argmax / logsumexp; the sampled row of cell_context
    is gathered back via indirect DMA to build the next query.
"""

import sys

if "/opt/trn_rl_repo" not in sys.path:
    sys.path.insert(0, "/opt/trn_rl_repo")

import numpy as np

B, S, E, H = 128, 1024, 128, 128
N_CORES = 8
BL = B // N_CORES  # 16 batch rows per core
N_STEPS = S - 1  # 1023
C_LOGIT = 10.0
NEG_INF = -1e9


# ---------------------------------------------------------------------------
# Host-side precompute (CPU jax — bitwise identical to the reference's
# loop-invariant computations and random draws)
# ---------------------------------------------------------------------------

def _host_precompute(cell_context, high_mask, init_w, Wc, bc, Wv, bv, W_ref, W_q,
                     n_steps=N_STEPS):
    import jax
    import jax.numpy as jnp

    cpu = jax.devices("cpu")[0]
    with jax.default_device(cpu):
        put = lambda x: jax.device_put(jnp.asarray(x), cpu)
        cc = put(cell_context)
        ref = jnp.einsum("bse,eh->bsh", cc, put(W_ref))  # [B, S, H]
        h_bar = jnp.mean(cc, axis=1) @ put(Wc) + put(bc)  # [B, E]
        q0 = h_bar + (put(init_w) @ put(Wv) + put(bv))  # [B, E]
        q0p = q0 @ put(W_q)  # projected first-step query [B, H]

        base_key = jax.random.key(42)

        @jax.jit
        def gum(i):
            return jax.random.gumbel(
                jax.random.fold_in(base_key, i), (B, S), jnp.float32
            )

        g = jax.lax.map(gum, jnp.arange(n_steps))  # [n_steps, B, S]

        ref = np.asarray(ref)
        h_bar = np.asarray(h_bar)
        q0p = np.asarray(q0p)
        g = np.asarray(g)

    mask0 = np.asarray(high_mask).copy()
    mask0[:, 0] = 1.0
    maskadd0 = np.where(mask0 > 0, np.float32(NEG_INF), np.float32(0.0)).astype(
        np.float32
    )
    return ref, h_bar, q0p, g, maskadd0


# ---------------------------------------------------------------------------
# Bass kernel builder
# ---------------------------------------------------------------------------

def build_bass_kernel(n_steps=N_STEPS, strip=(), static=False, unroll=1):
    strip = set(strip)
    import concourse.bass as bass
    import concourse.bacc as bacc
    import concourse.mybir as mybir
    from concourse.bass import ds
    from concourse.masks import make_identity
    from concourse.tile import TileContext

    dt = mybir.dt
    AF = mybir.ActivationFunctionType
    OP = mybir.AluOpType
    f32 = dt.float32

    nc = bacc.Bacc(
        "TRN2", target_bir_lowering=False, debug=False, num_devices=N_CORES
    )

    # --- DRAM I/O ---
    refT_d = nc.declare_dram_parameter("refT", [H, BL * S], f32, isOutput=False)
    g_d = nc.declare_dram_parameter("g_all", [n_steps * BL, S], f32, isOutput=False)
    cc_d = nc.declare_dram_parameter("cc_rows", [BL * S, E], f32, isOutput=False)
    qT0_d = nc.declare_dram_parameter("qT0", [H, BL], f32, isOutput=False)
    hbarT_d = nc.declare_dram_parameter("h_barT", [E, BL], f32, isOutput=False)
    bvT_d = nc.declare_dram_parameter("bvT", [E, 1], f32, isOutput=False)
    Wv1_d = nc.declare_dram_parameter("Wv1", [E, E], f32, isOutput=False)
    Wv2_d = nc.declare_dram_parameter("Wv2", [E, E], f32, isOutput=False)
    Wq_d = nc.declare_dram_parameter("Wq", [E, H], f32, isOutput=False)
    v_d = nc.declare_dram_parameter("v_col", [H, 1], f32, isOutput=False)
    maskadd0_d = nc.declare_dram_parameter("maskadd0", [BL, S], f32, isOutput=False)
    iota_d = nc.declare_dram_parameter("iota_s", [BL, S], f32, isOutput=False)
    bbase_d = nc.declare_dram_parameter("bbase", [BL, 1], dt.uint32, isOutput=False)
    pidx_d = nc.declare_dram_parameter("pidx", [BL, 1], f32, isOutput=False)
    idx_out_d = nc.declare_dram_parameter("idx_out", [BL, n_steps], dt.uint32,
                                          isOutput=True)
    logp_out_d = nc.declare_dram_parameter("logp_out", [BL, n_steps], f32,
                                           isOutput=True)

    with TileContext(nc) as tc:
        from contextlib import ExitStack

        with ExitStack() as ctx:
            const = ctx.enter_context(tc.tile_pool(name="const", bufs=1))
            work = ctx.enter_context(tc.tile_pool(name="work", bufs=1))
            tpool = ctx.enter_context(tc.tile_pool(name="tch", bufs=2))
            upsum = ctx.enter_context(tc.tile_pool(name="upsum", bufs=2, space="PSUM"))
            spsum = ctx.enter_context(tc.tile_pool(name="spsum", bufs=1, space="PSUM"))

            # --- persistent SBUF tensors ---
            ref_sb = const.tile([H, BL * S], f32, tag="ref")
            Wv1_sb = const.tile([E, E], f32, tag="wv1")
            Wv2_sb = const.tile([E, E], f32, tag="wv2")
            Wq_sb = const.tile([E, H], f32, tag="wq")
            v_sb = const.tile([H, 1], f32, tag="v")
            v_rep_sb = const.tile([H, 128], f32, tag="vrep")
            bvT_sb = const.tile([E, 1], f32, tag="bvt")
            hbarT_sb = const.tile([E, BL], f32, tag="hbart")
            iota_sb = const.tile([BL, S], f32, tag="iota")
            bbase_sb = const.tile([BL, 1], dt.uint32, tag="bbase")
            pidx_sb = const.tile([BL, 1], f32, tag="pidx")
            rmask_sb = const.tile([BL, BL], dt.uint8, tag="rmask")
            ident_sb = const.tile([128, 128], f32, tag="ident")

            maskadd_sb = const.tile([BL, S], f32, tag="maskadd")
            qT_sb = const.tile([H, BL], f32, tag="qt")
            queryT_sb = const.tile([E, BL], f32, tag="queryt")
            inithT_sb = const.tile([E, BL], f32, tag="initht")
            hT_sb = const.tile([E, BL], f32, tag="ht")

            # --- per-step scratch (allocated once, reused each iteration) ---
            g_sb = work.tile([BL, S], f32, tag="g")
            u_sb = work.tile([BL, S], f32, tag="u")
            th_sb = work.tile([BL, S], f32, tag="th")
            logits_sb = work.tile([BL, S], f32, tag="logits")
            y_sb = work.tile([BL, S], f32, tag="y")
            exp_sb = work.tile([BL, S], f32, tag="exp")
            eq_sb = work.tile([BL, S], f32, tag="eq")
            junk_sb = work.tile([BL, S], f32, tag="junk")
            max8_sb = work.tile([BL, 8], f32, tag="max8")
            idx8_sb = work.tile([BL, 8], dt.uint32, tag="idx8")
            m_sb = work.tile([BL, 1], f32, tag="m")
            negm_sb = work.tile([BL, 1], f32, tag="negm")
            sumexp_sb = work.tile([BL, 1], f32, tag="sumexp")
            lse_sb = work.tile([BL, 1], f32, tag="lse")
            usel_sb = work.tile([BL, 1], f32, tag="usel")
            logp_sb = work.tile([BL, 1], f32, tag="logp")
            idxf_sb = work.tile([BL, 1], f32, tag="idxf")
            idxg_sb = work.tile([BL, 1], dt.uint32, tag="idxg")
            h_rows_sb = work.tile([BL, E], f32, tag="hrows")

            # --- init loads ---
            nc.sync.dma_start(ref_sb[:], refT_d[:])
            nc.sync.dma_start(Wv1_sb[:], Wv1_d[:])
            nc.sync.dma_start(Wv2_sb[:], Wv2_d[:])
            nc.sync.dma_start(Wq_sb[:], Wq_d[:])
            nc.sync.dma_start(v_sb[:], v_d[:])
            nc.sync.dma_start(bvT_sb[:], bvT_d[:])
            nc.sync.dma_start(hbarT_sb[:], hbarT_d[:])
            nc.sync.dma_start(iota_sb[:], iota_d[:])
            nc.sync.dma_start(bbase_sb[:], bbase_d[:])
            nc.sync.dma_start(maskadd_sb[:], maskadd0_d[:])
            nc.sync.dma_start(qT_sb[:], qT0_d[:])
            make_identity(nc, ident_sb[:])
            nc.sync.dma_start(pidx_sb[:], pidx_d[:])
            nc.vector.tensor_copy(v_rep_sb[:], v_sb[:, 0:1].to_broadcast([H, 128]))
            for b in range(BL):
                nc.vector.tensor_scalar(
                    rmask_sb[:, b:b + 1], pidx_sb[:], float(b), None,
                    op0=OP.is_equal,
                )
            # Collapse the init-DMA fan-in to one semaphore: engine
            # instructions (ACT in particular) only have a single wait slot.
            tc.strict_bb_all_engine_barrier()

            def body(i, first):
                if "dyn" in strip and not isinstance(i, int):
                    i = 0
                # Gumbel slab for this step
                nc.sync.dma_start(g_sb[:], g_d[ds(i * BL, BL), :])

                # --- big stage: t = tanh(ref + q), u = sum_h v*t ---
                for b in range(BL):
                    tch = tpool.tile([H, S], f32, tag="t")
                    nc.scalar.activation(
                        tch[:],
                        ref_sb[:, b * S:(b + 1) * S],
                        AF.Tanh,
                        bias=qT_sb[:, b:b + 1],
                        scale=1.0,
                    )
                    ups = upsum.tile([128, S], f32, tag="ups")
                    nc.tensor.matmul(
                        ups[:, 0:512], lhsT=v_rep_sb[:], rhs=tch[:, 0:512],
                        start=True, stop=True,
                    )
                    nc.tensor.matmul(
                        ups[:, 512:1024], lhsT=v_rep_sb[:], rhs=tch[:, 512:1024],
                        start=True, stop=True,
                    )
                    # u is replicated across all PSUM partitions; a predicated
                    # copy lands it on row b only (DVE slices must start at a
                    # 32-aligned partition, so a direct [b:b+1] copy is out).
                    nc.vector.copy_predicated(
                        u_sb[:],
                        rmask_sb[:, b:b + 1].to_broadcast([BL, S]),
                        ups[0:BL, :],
                    )

                # --- logits = C*tanh(u) + maskadd ---
                nc.scalar.activation(th_sb[:], u_sb[:], AF.Tanh)
                nc.vector.scalar_tensor_tensor(
                    logits_sb[:], in0=th_sb[:], scalar=C_LOGIT, in1=maskadd_sb[:],
                    op0=OP.mult, op1=OP.add,
                )

                # --- gumbel argmax ---
                nc.vector.tensor_tensor(y_sb[:], logits_sb[:], g_sb[:], op=OP.add)
                if "max" in strip:
                    nc.vector.memset(max8_sb[:], 0.0)
                    nc.vector.memset(idx8_sb[:], 0)
                else:
                    nc.vector.max(max8_sb[:], y_sb[:])
                    nc.vector.max_index(idx8_sb[:], max8_sb[:], y_sb[:])

                # --- logp = logits[idx] - m - log(sum(exp(logits - m))) ---
                nc.vector.tensor_reduce(
                    m_sb[:], logits_sb[:], axis=mybir.AxisListType.X, op=OP.max
                )
                nc.vector.tensor_scalar_mul(negm_sb[:], m_sb[:], -1.0)
                nc.scalar.activation(
                    exp_sb[:], logits_sb[:], AF.Exp, bias=negm_sb[:, 0:1], scale=1.0,
                    accum_out=sumexp_sb[:],
                )
                nc.scalar.activation(lse_sb[:], sumexp_sb[:], AF.Ln)

                # one-hot of idx (f32 compare against iota)
                nc.vector.tensor_copy(idxf_sb[:], idx8_sb[:, 0:1])
                nc.vector.tensor_scalar(
                    eq_sb[:], iota_sb[:], idxf_sb[:, 0:1], None, op0=OP.is_equal
                )
                # select logits[idx] via masked sum (NOTE: tensor_tensor_reduce
                # is a custom-DVE op whose ucode table this runtime does not
                # load — it crashes the exec unit; use two standard ops)
                nc.vector.tensor_tensor(
                    junk_sb[:], logits_sb[:], eq_sb[:], op=OP.mult
                )
                nc.vector.tensor_reduce(
                    usel_sb[:], junk_sb[:], axis=mybir.AxisListType.X, op=OP.add
                )
                nc.vector.tensor_scalar(
                    logp_sb[:], usel_sb[:], m_sb[:, 0:1], lse_sb[:, 0:1],
                    op0=OP.subtract, op1=OP.subtract,
                )

                # --- outputs ---
                nc.sync.dma_start(idx_out_d[:, ds(i, 1)], idx8_sb[:, 0:1])
                nc.sync.dma_start(logp_out_d[:, ds(i, 1)], logp_sb[:, 0:1])

                # --- mask update (for next step) ---
                nc.vector.scalar_tensor_tensor(
                    maskadd_sb[:], in0=eq_sb[:], scalar=NEG_INF, in1=maskadd_sb[:],
                    op0=OP.mult, op1=OP.add,
                )

                # --- gather h = cc[b, idx_b, :] and build next query ---
                nc.vector.tensor_tensor(
                    idxg_sb[:], idx8_sb[:, 0:1], bbase_sb[:], op=OP.add
                )
                if "ind" in strip:
                    nc.sync.dma_start(h_rows_sb[:], cc_d[0:BL, :])
                else:
                    nc.gpsimd.indirect_dma_start(
                        out=h_rows_sb[:],
                        out_offset=None,
                        in_=cc_d[:],
                        in_offset=bass.IndirectOffsetOnAxis(
                            ap=idxg_sb[:, :1], axis=0
                        ),
                    )
                hT_ps = spsum.tile([E, BL], f32, tag="sp")
                nc.tensor.transpose(hT_ps[:], h_rows_sb[:], ident_sb[:BL, :BL])
                nc.vector.tensor_copy(hT_sb[:], hT_ps[:])
                if first:
                    nc.vector.tensor_copy(inithT_sb[:], hT_sb[:])

                q_ps = spsum.tile([E, BL], f32, tag="sp")
                nc.tensor.matmul(
                    q_ps[:], lhsT=Wv1_sb[:], rhs=inithT_sb[:], start=True, stop=False
                )
                nc.tensor.matmul(
                    q_ps[:], lhsT=Wv2_sb[:], rhs=hT_sb[:], start=False, stop=True
                )
                nc.vector.tensor_tensor(
                    queryT_sb[:], q_ps[:], hbarT_sb[:], op=OP.add
                )
                nc.vector.tensor_scalar(
                    queryT_sb[:], queryT_sb[:], bvT_sb[:, 0:1], None, op0=OP.add
                )
                qT_ps = spsum.tile([H, BL], f32, tag="sp")
                nc.tensor.matmul(
                    qT_ps[:], lhsT=Wq_sb[:], rhs=queryT_sb[:], start=True, stop=True
                )
                nc.vector.tensor_copy(qT_sb[:], qT_ps[:])

            body(0, True)
            if n_steps > 1:
                if static:
                    for i in range(1, n_steps):
                        body(i, False)
                elif unroll > 1:
                    assert (n_steps - 1) % unroll == 0, (n_steps, unroll)
                    with tc.For_i(1, n_steps, step=unroll) as iv:
                        for u in range(unroll):
                            body(iv + u, False)
                else:
                    with tc.For_i(1, n_steps) as iv:
                        body(iv, False)

    nc.compile()
    return nc


# ---------------------------------------------------------------------------
# Per-core input maps
# ---------------------------------------------------------------------------

def make_in_maps(cell_context, ref, h_bar, q0p, g, maskadd0, Wv, bv, W_q, v,
                 n_steps=N_STEPS):
    cc = np.ascontiguousarray(np.asarray(cell_context), dtype=np.float32)
    Wv = np.asarray(Wv, dtype=np.float32)
    bv = np.asarray(bv, dtype=np.float32)
    W_q = np.asarray(W_q, dtype=np.float32)
    v = np.asarray(v, dtype=np.float32)

    iota_s = np.broadcast_to(
        np.arange(S, dtype=np.float32)[None, :], (BL, S)
    ).copy()
    pidx = np.arange(BL, dtype=np.float32)[:, None].copy()
    bbase = (np.arange(BL, dtype=np.uint32) * S)[:, None].copy()
    Wv1 = np.ascontiguousarray(Wv[:E])
    Wv2 = np.ascontiguousarray(Wv[E:])
    bvT = np.ascontiguousarray(bv[:, None])
    v_col = np.ascontiguousarray(v[:, None])

    in_maps = []
    for c in range(N_CORES):
        sl = slice(c * BL, (c + 1) * BL)
        refT = np.ascontiguousarray(
            np.transpose(ref[sl], (2, 0, 1)).reshape(H, BL * S)
        )
        in_maps.append(
            {
                "refT": refT,
                "g_all": np.ascontiguousarray(
                    g[:n_steps, sl, :].reshape(n_steps * BL, S)
                ),
                "cc_rows": np.ascontiguousarray(cc[sl].reshape(BL * S, E)),
                "qT0": np.ascontiguousarray(q0p[sl].T),
                "h_barT": np.ascontiguousarray(h_bar[sl].T),
                "bvT": bvT,
                "Wv1": Wv1,
                "Wv2": Wv2,
                "Wq": W_q,
                "v_col": v_col,
                "maskadd0": np.ascontiguousarray(maskadd0[sl]),
                "iota_s": iota_s,
                "bbase": bbase,
                "pidx": pidx,
            }
        )
    return in_maps


_BUILD_CACHE = {}


def _get_kernel(n_steps=N_STEPS):
    if n_steps not in _BUILD_CACHE:
        _BUILD_CACHE[n_steps] = build_bass_kernel(n_steps)
    return _BUILD_CACHE[n_steps]


def run_on_hw(in_maps, n_steps=N_STEPS, trace=False):
    from concourse.bass_utils import run_bass_kernel_spmd

    nc = _get_kernel(n_steps)
    res = run_bass_kernel_spmd(
        nc, in_maps, core_ids=list(range(len(in_maps))), trace=trace
    )
    return res


def kernel(node_context, original_data, cell_context, high_mask, low_mask,
           init_w, Wc, bc, Wv, bv, W_ref, W_q, v):
    cell_context = np.asarray(cell_context)
    ref, h_bar, q0p, g, maskadd0 = _host_precompute(
        cell_context, high_mask, init_w, Wc, bc, Wv, bv, W_ref, W_q
    )
    in_maps = make_in_maps(
        cell_context, ref, h_bar, q0p, g, maskadd0, Wv, bv, W_q, v
    )
    res = run_on_hw(in_maps)
    logps = np.concatenate([r["logp_out"] for r in res.results], axis=0)
    idxs = np.concatenate(
        [r["idx_out"].astype(np.int32) for r in res.results], axis=0
    )
    return logps, idxs
